# revision 11
# baseline (speedup 1.0000x reference)
"""CoordinateLSTM cell on 8 Trainium2 NeuronCores (Bass/Tile, data-parallel).

Computes, for B=32768, I=H=128:
    total = concat([x, h], -1)                # [B, 256]
    s1 = sigmoid(total @ W1.T + b1)
    s2 = sigmoid(total @ W2.T + b2)
    fl = tanh   (total @ Wf.T + bf)
    s3 = sigmoid(total @ W3.T + b3)
    new_c = c * s1 + s2 * fl
    new_h = tanh(new_c) * s3

Sharding: batch dim split 8 ways (4096 rows/core); weights replicated.

Per-core kernel structure (per 1024-row chunk):
  - gpsimd (SWDGE) DMA loads x,h with an inline f32->bf16 cast (free cast)
  - HWDGE xbar DMA-transposes each 128-row subtile to feature-major layout
  - per 128-row subtile: 3 accumulating bf16 matmuls into one PSUM bank
    [128, 512]: xT.T@Wtx + hT.T@Wth + ones.T@bias (rank-1 bias add)
  - ScalarE sigmoid/tanh directly off PSUM (gates packed [s1|s2|s3|fl])
  - VectorE elementwise combine in f32, stores via HWDGE
"""

import sys

if "/opt/trn_rl_repo" not in sys.path:
    sys.path.insert(0, "/opt/trn_rl_repo")

import numpy as np
import ml_dtypes

BF16 = ml_dtypes.bfloat16
MM_DT = np.float16  # matmul operand dtype: fp16 = 10-bit mantissa, 1 cyc/row

B, I, H = 32768, 128, 128
N_CORES = 8
B_CORE = B // N_CORES  # 4096
SUB = 128              # rows per matmul tile (M)
G = 512                # stacked gate width: [s1 | s2 | s3 | fl]
SUBS_PER_GROUP = 4     # subtiles per PSUM group (4 banks)
GROUPS_PER_CHUNK = 2
CHUNK = SUB * SUBS_PER_GROUP * GROUPS_PER_CHUNK  # 1024 rows, load granularity

TRACE = False          # set by test.py to profile
LAST_EXEC_NS = None

_cache = {}


def _build(rows, reps=1, loop_n=1):
    """Build + compile the per-core Bass program for `rows` rows.

    reps > 1 unrolls the whole computation that many times; loop_n > 1 wraps
    it in a device-side For_i loop. Both are idempotent (same inputs/outputs)
    and exist so wall-clock differencing can recover the pure kernel
    execution time without NTFF profiling.
    """
    import concourse.bacc as bacc
    import concourse.bass as bass
    import concourse.tile as tile
    import concourse.mybir as mybir
    from contextlib import ExitStack, nullcontext

    dt = mybir.dt
    global MM_DT_BIR
    MM_DT_BIR = dt.float16 if MM_DT == np.float16 else dt.bfloat16
    AF = mybir.ActivationFunctionType
    assert rows % CHUNK == 0
    n_chunks = rows // CHUNK
    spc = CHUNK // SUB  # subtiles per chunk = 8

    nc = bacc.Bacc(
        "TRN2",
        target_bir_lowering=False,
        debug=False,
        enable_asserts=False,
        num_devices=N_CORES,
    )
    x_d = nc.dram_tensor("x", [rows, I], dt.float32, kind="ExternalInput")
    h_d = nc.dram_tensor("h", [rows, H], dt.float32, kind="ExternalInput")
    c_d = nc.dram_tensor("c", [rows, H], dt.float32, kind="ExternalInput")
    wtx_d = nc.dram_tensor("wtx", [I, G], MM_DT_BIR, kind="ExternalInput")
    wth_d = nc.dram_tensor("wth", [H, G], MM_DT_BIR, kind="ExternalInput")
    bias_d = nc.dram_tensor("bias", [1, G], MM_DT_BIR, kind="ExternalInput")
    ones_d = nc.dram_tensor("ones", [1, SUB], MM_DT_BIR, kind="ExternalInput")
    nh_d = nc.dram_tensor("new_h", [rows, H], dt.float32, kind="ExternalOutput")
    ncv_d = nc.dram_tensor("new_c", [rows, H], dt.float32, kind="ExternalOutput")

    # DRAM views: chunk-of-subtiles with rows-within-subtile on partitions.
    x_r = x_d[:].rearrange("(n s p) c -> n p s c", s=spc, p=SUB)
    h_r = h_d[:].rearrange("(n s p) c -> n p s c", s=spc, p=SUB)
    c_r = c_d[:].rearrange("(n s p) c -> n p s c", s=spc, p=SUB)
    nh_r = nh_d[:].rearrange(
        "(n g s p) c -> n g p s c", g=GROUPS_PER_CHUNK, s=SUBS_PER_GROUP, p=SUB
    )
    ncv_r = ncv_d[:].rearrange(
        "(n g s p) c -> n g p s c", g=GROUPS_PER_CHUNK, s=SUBS_PER_GROUP, p=SUB
    )

    with tile.TileContext(nc) as tc, ExitStack() as ctx:
        const = ctx.enter_context(tc.tile_pool(name="const", bufs=1))
        wtx_sb = const.tile([I, G], MM_DT_BIR)
        nc.sync.dma_start(wtx_sb[:], wtx_d[:])
        wth_sb = const.tile([H, G], MM_DT_BIR)
        nc.sync.dma_start(wth_sb[:], wth_d[:])
        bias_sb = const.tile([1, G], MM_DT_BIR)
        nc.sync.dma_start(bias_sb[:], bias_d[:])
        ones_sb = const.tile([1, SUB], MM_DT_BIR)
        nc.sync.dma_start(ones_sb[:], ones_d[:])

        xin = ctx.enter_context(tc.tile_pool(name="xin", bufs=2))
        xtp = ctx.enter_context(tc.tile_pool(name="xtp", bufs=2))
        cin = ctx.enter_context(tc.tile_pool(name="cin", bufs=2))
        psum = ctx.enter_context(
            tc.tile_pool(name="psum", bufs=2, space=bass.MemorySpace.PSUM)
        )
        sigp = ctx.enter_context(tc.tile_pool(name="sigp", bufs=3))
        post = ctx.enter_context(tc.tile_pool(name="post", bufs=3))

        loop_cm = tc.For_i(0, loop_n, 1) if loop_n > 1 else nullcontext()
        with loop_cm:
         for _rep in range(reps):
          for n in range(n_chunks):
            xbf = xin.tile([SUB, spc, I], MM_DT_BIR, tag="xbf")
            nc.gpsimd.dma_start(xbf[:], x_r[n])  # f32 -> bf16 cast in DMA
            hbf = xin.tile([SUB, spc, H], MM_DT_BIR, tag="hbf")
            nc.gpsimd.dma_start(hbf[:], h_r[n])
            c_sb = cin.tile([SUB, spc, H], dt.float32, tag="c")
            nc.sync.dma_start(c_sb[:], c_r[n])

            xT = xtp.tile([I, spc, SUB], MM_DT_BIR, tag="xT")
            hT = xtp.tile([H, spc, SUB], MM_DT_BIR, tag="hT")
            for s in range(spc):
                nc.scalar.dma_start(xT[:, s, :], xbf[:, s, :], transpose=True)
                nc.scalar.dma_start(hT[:, s, :], hbf[:, s, :], transpose=True)

            for g in range(GROUPS_PER_CHUNK):
                ps = psum.tile([SUB, SUBS_PER_GROUP, G], dt.float32, tag="ps")
                for i in range(SUBS_PER_GROUP):
                    s = g * SUBS_PER_GROUP + i
                    nc.tensor.matmul(
                        ps[:, i, :], xT[:, s, :], wtx_sb[:], start=True, stop=False
                    )
                    nc.tensor.matmul(
                        ps[:, i, :], hT[:, s, :], wth_sb[:], start=False, stop=False
                    )
                    nc.tensor.matmul(
                        ps[:, i, :], ones_sb[:], bias_sb[:], start=False, stop=True
                    )

                sig = sigp.tile([SUB, SUBS_PER_GROUP, G], dt.float32, tag="sig")
                nc.scalar.activation(sig[:, :, 0:384], ps[:, :, 0:384], AF.Sigmoid)
                nc.scalar.activation(sig[:, :, 384:512], ps[:, :, 384:512], AF.Tanh)

                c_sl = c_sb[:, g * SUBS_PER_GROUP : (g + 1) * SUBS_PER_GROUP, :]
                m1 = post.tile([SUB, SUBS_PER_GROUP, H], dt.float32, tag="m1")
                nc.vector.tensor_mul(m1[:], c_sl, sig[:, :, 0:128])
                m2 = post.tile([SUB, SUBS_PER_GROUP, H], dt.float32, tag="m2")
                nc.vector.tensor_mul(m2[:], sig[:, :, 128:256], sig[:, :, 384:512])
                ncw = post.tile([SUB, SUBS_PER_GROUP, H], dt.float32, tag="ncw")
                nc.vector.tensor_add(ncw[:], m1[:], m2[:])
                th = post.tile([SUB, SUBS_PER_GROUP, H], dt.float32, tag="th")
                nc.scalar.activation(th[:], ncw[:], AF.Tanh)
                nhw = post.tile([SUB, SUBS_PER_GROUP, H], dt.float32, tag="nhw")
                nc.vector.tensor_mul(nhw[:], th[:], sig[:, :, 256:384])

                nc.sync.dma_start(ncv_r[n, g], ncw[:])
                nc.sync.dma_start(nh_r[n, g], nhw[:])

    nc.compile()
    return nc


def _get_program(rows):
    if rows not in _cache:
        _cache[rows] = _build(rows)
    return _cache[rows]


def _host_prep(W1, b1, W2, b2, Wf, bf, W3, b3):
    # Gate packing along the 512-wide output dim: [s1 | s2 | s3 | fl] so the
    # three sigmoid gates are contiguous for one ScalarE op.
    wtx = np.concatenate(
        [W1[:, :I].T, W2[:, :I].T, W3[:, :I].T, Wf[:, :I].T], axis=1
    ).astype(MM_DT)
    wth = np.concatenate(
        [W1[:, I:].T, W2[:, I:].T, W3[:, I:].T, Wf[:, I:].T], axis=1
    ).astype(MM_DT)
    bias = np.concatenate([b1, b2, b3, bf]).reshape(1, G).astype(MM_DT)
    ones = np.ones((1, SUB), MM_DT)
    return wtx, wth, bias, ones


def _make_runner(nc):
    """Cached jitted SPMD executor for `nc` (mirrors bass2jax.run_bass_via_pjrt
    but without output-buffer donation so device-resident inputs can be reused
    across timing calls)."""
    import jax
    import concourse.mybir as mybir
    from jax.experimental.shard_map import shard_map
    from jax.sharding import Mesh, PartitionSpec
    from concourse.bass2jax import (
        _bass_exec_p,
        install_neuronx_cc_hook,
        partition_id_tensor,
    )

    install_neuronx_cc_hook()
    assert nc.dbg_addr is None
    partition_name = nc.partition_id_tensor.name if nc.partition_id_tensor else None

    in_names, out_names, out_avals, zero_outs = [], [], [], []
    for alloc in nc.m.functions[0].allocations:
        if not isinstance(alloc, mybir.MemoryLocationSet):
            continue
        name = alloc.memorylocations[0].name
        if alloc.kind == "ExternalInput":
            if name != partition_name:
                in_names.append(name)
        elif alloc.kind == "ExternalOutput":
            out_names.append(name)
            shape = tuple(alloc.tensor_shape)
            dtype = mybir.dt.np(alloc.dtype)
            out_avals.append(jax.core.ShapedArray(shape, dtype))
            zero_outs.append(np.zeros(shape, dtype))
    n_params = len(in_names)
    all_names = in_names + out_names
    if partition_name is not None:
        all_names = all_names + [partition_name]

    def _body(*args):
        operands = list(args)
        if partition_name is not None:
            operands.append(partition_id_tensor())
        outs = _bass_exec_p.bind(
            *operands,
            out_avals=tuple(out_avals),
            in_names=tuple(all_names),
            out_names=tuple(out_names),
            lowering_input_output_aliases=(),
            sim_require_finite=True,
            sim_require_nnan=True,
            nc=nc,
        )
        return tuple(outs)

    devices = jax.devices()[:N_CORES]
    mesh = Mesh(np.asarray(devices), ("core",))
    n_all = n_params + len(out_names)
    sharded = jax.jit(
        shard_map(
            _body,
            mesh=mesh,
            in_specs=(PartitionSpec("core"),) * n_all,
            out_specs=(PartitionSpec("core"),) * len(out_names),
            check_rep=False,
        ),
        keep_unused=True,
    )
    return sharded, in_names, out_names, zero_outs


def _stage_inputs(in_maps, in_names, zero_outs):
    import jax

    concat_in = [
        np.concatenate([m[name] for m in in_maps], axis=0) for name in in_names
    ]
    concat_zeros = [
        np.zeros((N_CORES * z.shape[0], *z.shape[1:]), z.dtype) for z in zero_outs
    ]
    return [jax.device_put(a) for a in concat_in + concat_zeros]


def bench(
    x, h, c, W1, b1, W2, b2, Wf, bf, W3, b3, loop_lo=2048, loop_hi=6144, n_calls=4
):
    """Measure per-invocation HW time via wall-clock differencing between two
    device-side-looped builds (loop_lo vs loop_hi iterations), which cancels
    the per-call dispatch overhead. Returns (kernel_ns, tlo_list, thi_list)."""
    import time as _time

    import jax

    x = np.ascontiguousarray(x, np.float32)
    h = np.ascontiguousarray(h, np.float32)
    c = np.ascontiguousarray(c, np.float32)
    wtx, wth, bias, ones = _host_prep(W1, b1, W2, b2, Wf, bf, W3, b3)
    rows = x.shape[0] // N_CORES
    in_maps = []
    for k in range(N_CORES):
        sl = slice(k * rows, (k + 1) * rows)
        in_maps.append(
            dict(x=x[sl], h=h[sl], c=c[sl], wtx=wtx, wth=wth, bias=bias, ones=ones)
        )

    results = {}
    for loop_n in (loop_lo, loop_hi):
        nc = _build(rows, loop_n=loop_n)
        sharded, in_names, out_names, zero_outs = _make_runner(nc)
        dev_args = _stage_inputs(in_maps, in_names, zero_outs)
        outs = sharded(*dev_args)  # warmup/compile
        jax.block_until_ready(outs)
        times = []
        for _ in range(n_calls):
            t0 = _time.perf_counter()
            outs = sharded(*dev_args)
            jax.block_until_ready(outs)
            times.append((_time.perf_counter() - t0) * 1e9)
        results[loop_n] = times
    tlo = min(results[loop_lo])
    thi = min(results[loop_hi])
    kernel_ns = (thi - tlo) / (loop_hi - loop_lo)
    return kernel_ns, results[loop_lo], results[loop_hi]


def kernel(x, h, c, W1, b1, W2, b2, Wf, bf, W3, b3):
    from concourse.bass_utils import run_bass_kernel_spmd

    global LAST_EXEC_NS
    x = np.ascontiguousarray(x, np.float32)
    h = np.ascontiguousarray(h, np.float32)
    c = np.ascontiguousarray(c, np.float32)
    wtx, wth, bias, ones = _host_prep(W1, b1, W2, b2, Wf, bf, W3, b3)

    rows = x.shape[0] // N_CORES
    nc = _get_program(rows)

    in_maps = []
    for k in range(N_CORES):
        sl = slice(k * rows, (k + 1) * rows)
        in_maps.append(
            {
                "x": x[sl],
                "h": h[sl],
                "c": c[sl],
                "wtx": wtx,
                "wth": wth,
                "bias": bias,
                "ones": ones,
            }
        )

    res = run_bass_kernel_spmd(
        nc, in_maps, core_ids=list(range(N_CORES)), trace=TRACE
    )
    LAST_EXEC_NS = res.exec_time_ns

    new_h = np.concatenate([res.results[k]["new_h"] for k in range(N_CORES)], axis=0)
    new_c = np.concatenate([res.results[k]["new_c"] for k in range(N_CORES)], axis=0)
    return new_h, new_c


# revision 14
# speedup vs baseline: 1.1413x; 1.1413x over previous
"""CoordinateLSTM cell on 8 Trainium2 NeuronCores (Bass/Tile, data-parallel).

Computes, for B=32768, I=H=128:
    total = concat([x, h], -1)                # [B, 256]
    s1 = sigmoid(total @ W1.T + b1)
    s2 = sigmoid(total @ W2.T + b2)
    fl = tanh   (total @ Wf.T + bf)
    s3 = sigmoid(total @ W3.T + b3)
    new_c = c * s1 + s2 * fl
    new_h = tanh(new_c) * s3

Sharding: batch dim split 8 ways (4096 rows/core); weights replicated.

Per-core kernel structure (per 1024-row chunk):
  - gpsimd (SWDGE) DMA loads x,h with an inline f32->bf16 cast (free cast)
  - HWDGE xbar DMA-transposes each 128-row subtile to feature-major layout
  - per 128-row subtile: 3 accumulating bf16 matmuls into one PSUM bank
    [128, 512]: xT.T@Wtx + hT.T@Wth + ones.T@bias (rank-1 bias add)
  - ScalarE sigmoid/tanh directly off PSUM (gates packed [s1|s2|s3|fl])
  - VectorE elementwise combine in f32, stores via HWDGE
"""

import sys

if "/opt/trn_rl_repo" not in sys.path:
    sys.path.insert(0, "/opt/trn_rl_repo")

import numpy as np
import ml_dtypes

BF16 = ml_dtypes.bfloat16
MM_DT = np.float16  # matmul operand dtype: fp16 = 10-bit mantissa, 1 cyc/row

B, I, H = 32768, 128, 128
N_CORES = 8
B_CORE = B // N_CORES  # 4096
SUB = 128              # rows per matmul tile (M)
G = 512                # stacked gate width: [s1 | s2 | s3 | fl]
SUBS_PER_GROUP = 4     # subtiles per PSUM group (4 banks)
GROUPS_PER_CHUNK = 2
CHUNK = SUB * SUBS_PER_GROUP * GROUPS_PER_CHUNK  # 1024 rows, load granularity

TRACE = False          # set by test.py to profile
LAST_EXEC_NS = None

_cache = {}


def _build(rows, reps=1, loop_n=1):
    """Build + compile the per-core Bass program for `rows` rows.

    reps > 1 unrolls the whole computation that many times; loop_n > 1 wraps
    it in a device-side For_i loop. Both are idempotent (same inputs/outputs)
    and exist so wall-clock differencing can recover the pure kernel
    execution time without NTFF profiling.
    """
    import concourse.bacc as bacc
    import concourse.bass as bass
    import concourse.tile as tile
    import concourse.mybir as mybir
    from contextlib import ExitStack, nullcontext

    dt = mybir.dt
    global MM_DT_BIR
    MM_DT_BIR = dt.float16 if MM_DT == np.float16 else dt.bfloat16
    AF = mybir.ActivationFunctionType
    assert rows % CHUNK == 0
    n_chunks = rows // CHUNK
    spc = CHUNK // SUB  # subtiles per chunk = 8

    nc = bacc.Bacc(
        "TRN2",
        target_bir_lowering=False,
        debug=False,
        enable_asserts=False,
        num_devices=N_CORES,
    )
    x_d = nc.dram_tensor("x", [rows, I], dt.float32, kind="ExternalInput")
    h_d = nc.dram_tensor("h", [rows, H], dt.float32, kind="ExternalInput")
    c_d = nc.dram_tensor("c", [rows, H], dt.float32, kind="ExternalInput")
    wtx_d = nc.dram_tensor("wtx", [I, G], MM_DT_BIR, kind="ExternalInput")
    wth_d = nc.dram_tensor("wth", [H, G], MM_DT_BIR, kind="ExternalInput")
    bias_d = nc.dram_tensor("bias", [1, G], MM_DT_BIR, kind="ExternalInput")
    ones_d = nc.dram_tensor("ones", [1, SUB], MM_DT_BIR, kind="ExternalInput")
    nh_d = nc.dram_tensor("new_h", [rows, H], dt.float32, kind="ExternalOutput")
    ncv_d = nc.dram_tensor("new_c", [rows, H], dt.float32, kind="ExternalOutput")

    # DRAM slab views: partition p holds `spc` CONSECUTIVE rows (contiguous
    # 4 KiB per partition -> 1 DMA descriptor per partition instead of 8).
    # Logical subtile r of a chunk is the strided row set {spc*p + r}; the
    # same mapping is applied to x, h, c and the outputs, so the matmul /
    # elementwise / store row-identity stays consistent.
    x_r = x_d[:].rearrange("(n p r) c -> n p r c", r=spc, p=SUB)
    h_r = h_d[:].rearrange("(n p r) c -> n p r c", r=spc, p=SUB)
    c_r = c_d[:].rearrange("(n p r) c -> n p r c", r=spc, p=SUB)
    nh_r = nh_d[:].rearrange("(n p r) c -> n p r c", r=spc, p=SUB)
    ncv_r = ncv_d[:].rearrange("(n p r) c -> n p r c", r=spc, p=SUB)

    with tile.TileContext(nc) as tc, ExitStack() as ctx:
        const = ctx.enter_context(tc.tile_pool(name="const", bufs=1))
        wtx_sb = const.tile([I, G], MM_DT_BIR)
        nc.sync.dma_start(wtx_sb[:], wtx_d[:])
        wth_sb = const.tile([H, G], MM_DT_BIR)
        nc.sync.dma_start(wth_sb[:], wth_d[:])
        bias_sb = const.tile([1, G], MM_DT_BIR)
        nc.sync.dma_start(bias_sb[:], bias_d[:])
        ones_sb = const.tile([1, SUB], MM_DT_BIR)
        nc.sync.dma_start(ones_sb[:], ones_d[:])

        xin = ctx.enter_context(tc.tile_pool(name="xin", bufs=2))
        xtp = ctx.enter_context(tc.tile_pool(name="xtp", bufs=2))
        cin = ctx.enter_context(tc.tile_pool(name="cin", bufs=2))
        psum = ctx.enter_context(
            tc.tile_pool(name="psum", bufs=2, space=bass.MemorySpace.PSUM)
        )
        sigp = ctx.enter_context(tc.tile_pool(name="sigp", bufs=3))
        post = ctx.enter_context(tc.tile_pool(name="post", bufs=3))

        loop_cm = tc.For_i(0, loop_n, 1) if loop_n > 1 else nullcontext()
        with loop_cm:
         for _rep in range(reps):
          for n in range(n_chunks):
            xbf = xin.tile([SUB, spc, I], MM_DT_BIR, tag="xbf")
            nc.gpsimd.dma_start(xbf[:], x_r[n])  # f32 -> bf16 cast in DMA
            hbf = xin.tile([SUB, spc, H], MM_DT_BIR, tag="hbf")
            nc.gpsimd.dma_start(hbf[:], h_r[n])
            c_sb = cin.tile([SUB, spc, H], dt.float32, tag="c")
            nc.sync.dma_start(c_sb[:], c_r[n])

            xT = xtp.tile([I, spc, SUB], MM_DT_BIR, tag="xT")
            hT = xtp.tile([H, spc, SUB], MM_DT_BIR, tag="hT")
            for s in range(spc):
                nc.scalar.dma_start(xT[:, s, :], xbf[:, s, :], transpose=True)
                nc.scalar.dma_start(hT[:, s, :], hbf[:, s, :], transpose=True)

            ncw = post.tile([SUB, spc, H], dt.float32, tag="ncw")
            nhw = post.tile([SUB, spc, H], dt.float32, tag="nhw")
            for g in range(GROUPS_PER_CHUNK):
                ps = psum.tile([SUB, SUBS_PER_GROUP, G], dt.float32, tag="ps")
                for i in range(SUBS_PER_GROUP):
                    s = g * SUBS_PER_GROUP + i
                    nc.tensor.matmul(
                        ps[:, i, :], xT[:, s, :], wtx_sb[:], start=True, stop=False
                    )
                    nc.tensor.matmul(
                        ps[:, i, :], hT[:, s, :], wth_sb[:], start=False, stop=False
                    )
                    nc.tensor.matmul(
                        ps[:, i, :], ones_sb[:], bias_sb[:], start=False, stop=True
                    )

                sig = sigp.tile([SUB, SUBS_PER_GROUP, G], dt.float32, tag="sig")
                nc.scalar.activation(sig[:, :, 0:384], ps[:, :, 0:384], AF.Sigmoid)
                nc.scalar.activation(sig[:, :, 384:512], ps[:, :, 384:512], AF.Tanh)

                gsl = slice(g * SUBS_PER_GROUP, (g + 1) * SUBS_PER_GROUP)
                c_sl = c_sb[:, gsl, :]
                ncw_sl = ncw[:, gsl, :]
                nhw_sl = nhw[:, gsl, :]
                m1 = post.tile([SUB, SUBS_PER_GROUP, H], dt.float32, tag="m1")
                nc.vector.tensor_mul(m1[:], c_sl, sig[:, :, 0:128])
                m2 = post.tile([SUB, SUBS_PER_GROUP, H], dt.float32, tag="m2")
                nc.vector.tensor_mul(m2[:], sig[:, :, 128:256], sig[:, :, 384:512])
                nc.vector.tensor_add(ncw_sl, m1[:], m2[:])
                th = post.tile([SUB, SUBS_PER_GROUP, H], dt.float32, tag="th")
                nc.scalar.activation(th[:], ncw_sl, AF.Tanh)
                nc.vector.tensor_mul(nhw_sl, th[:], sig[:, :, 256:384])

            nc.sync.dma_start(ncv_r[n], ncw[:])
            nc.sync.dma_start(nh_r[n], nhw[:])

    nc.compile()
    return nc


def _get_program(rows):
    if rows not in _cache:
        _cache[rows] = _build(rows)
    return _cache[rows]


def _host_prep(W1, b1, W2, b2, Wf, bf, W3, b3):
    # Gate packing along the 512-wide output dim: [s1 | s2 | s3 | fl] so the
    # three sigmoid gates are contiguous for one ScalarE op.
    wtx = np.concatenate(
        [W1[:, :I].T, W2[:, :I].T, W3[:, :I].T, Wf[:, :I].T], axis=1
    ).astype(MM_DT)
    wth = np.concatenate(
        [W1[:, I:].T, W2[:, I:].T, W3[:, I:].T, Wf[:, I:].T], axis=1
    ).astype(MM_DT)
    bias = np.concatenate([b1, b2, b3, bf]).reshape(1, G).astype(MM_DT)
    ones = np.ones((1, SUB), MM_DT)
    return wtx, wth, bias, ones


def _make_runner(nc):
    """Cached jitted SPMD executor for `nc` (mirrors bass2jax.run_bass_via_pjrt
    but without output-buffer donation so device-resident inputs can be reused
    across timing calls)."""
    import jax
    import concourse.mybir as mybir
    from jax.experimental.shard_map import shard_map
    from jax.sharding import Mesh, PartitionSpec
    from concourse.bass2jax import (
        _bass_exec_p,
        install_neuronx_cc_hook,
        partition_id_tensor,
    )

    install_neuronx_cc_hook()
    assert nc.dbg_addr is None
    partition_name = nc.partition_id_tensor.name if nc.partition_id_tensor else None

    in_names, out_names, out_avals, zero_outs = [], [], [], []
    for alloc in nc.m.functions[0].allocations:
        if not isinstance(alloc, mybir.MemoryLocationSet):
            continue
        name = alloc.memorylocations[0].name
        if alloc.kind == "ExternalInput":
            if name != partition_name:
                in_names.append(name)
        elif alloc.kind == "ExternalOutput":
            out_names.append(name)
            shape = tuple(alloc.tensor_shape)
            dtype = mybir.dt.np(alloc.dtype)
            out_avals.append(jax.core.ShapedArray(shape, dtype))
            zero_outs.append(np.zeros(shape, dtype))
    n_params = len(in_names)
    all_names = in_names + out_names
    if partition_name is not None:
        all_names = all_names + [partition_name]

    def _body(*args):
        operands = list(args)
        if partition_name is not None:
            operands.append(partition_id_tensor())
        outs = _bass_exec_p.bind(
            *operands,
            out_avals=tuple(out_avals),
            in_names=tuple(all_names),
            out_names=tuple(out_names),
            lowering_input_output_aliases=(),
            sim_require_finite=True,
            sim_require_nnan=True,
            nc=nc,
        )
        return tuple(outs)

    devices = jax.devices()[:N_CORES]
    mesh = Mesh(np.asarray(devices), ("core",))
    n_all = n_params + len(out_names)
    sharded = jax.jit(
        shard_map(
            _body,
            mesh=mesh,
            in_specs=(PartitionSpec("core"),) * n_all,
            out_specs=(PartitionSpec("core"),) * len(out_names),
            check_rep=False,
        ),
        keep_unused=True,
    )
    return sharded, in_names, out_names, zero_outs


def _stage_inputs(in_maps, in_names, zero_outs):
    import jax

    concat_in = [
        np.concatenate([m[name] for m in in_maps], axis=0) for name in in_names
    ]
    concat_zeros = [
        np.zeros((N_CORES * z.shape[0], *z.shape[1:]), z.dtype) for z in zero_outs
    ]
    return [jax.device_put(a) for a in concat_in + concat_zeros]


def bench(
    x, h, c, W1, b1, W2, b2, Wf, bf, W3, b3, loop_lo=2048, loop_hi=6144, n_calls=4
):
    """Measure per-invocation HW time via wall-clock differencing between two
    device-side-looped builds (loop_lo vs loop_hi iterations), which cancels
    the per-call dispatch overhead. Returns (kernel_ns, tlo_list, thi_list)."""
    import time as _time

    import jax

    x = np.ascontiguousarray(x, np.float32)
    h = np.ascontiguousarray(h, np.float32)
    c = np.ascontiguousarray(c, np.float32)
    wtx, wth, bias, ones = _host_prep(W1, b1, W2, b2, Wf, bf, W3, b3)
    rows = x.shape[0] // N_CORES
    in_maps = []
    for k in range(N_CORES):
        sl = slice(k * rows, (k + 1) * rows)
        in_maps.append(
            dict(x=x[sl], h=h[sl], c=c[sl], wtx=wtx, wth=wth, bias=bias, ones=ones)
        )

    results = {}
    for loop_n in (loop_lo, loop_hi):
        nc = _build(rows, loop_n=loop_n)
        sharded, in_names, out_names, zero_outs = _make_runner(nc)
        dev_args = _stage_inputs(in_maps, in_names, zero_outs)
        outs = sharded(*dev_args)  # warmup/compile
        jax.block_until_ready(outs)
        times = []
        for _ in range(n_calls):
            t0 = _time.perf_counter()
            outs = sharded(*dev_args)
            jax.block_until_ready(outs)
            times.append((_time.perf_counter() - t0) * 1e9)
        results[loop_n] = times
    tlo = min(results[loop_lo])
    thi = min(results[loop_hi])
    kernel_ns = (thi - tlo) / (loop_hi - loop_lo)
    return kernel_ns, results[loop_lo], results[loop_hi]


def kernel(x, h, c, W1, b1, W2, b2, Wf, bf, W3, b3):
    from concourse.bass_utils import run_bass_kernel_spmd

    global LAST_EXEC_NS
    x = np.ascontiguousarray(x, np.float32)
    h = np.ascontiguousarray(h, np.float32)
    c = np.ascontiguousarray(c, np.float32)
    wtx, wth, bias, ones = _host_prep(W1, b1, W2, b2, Wf, bf, W3, b3)

    rows = x.shape[0] // N_CORES
    nc = _get_program(rows)

    in_maps = []
    for k in range(N_CORES):
        sl = slice(k * rows, (k + 1) * rows)
        in_maps.append(
            {
                "x": x[sl],
                "h": h[sl],
                "c": c[sl],
                "wtx": wtx,
                "wth": wth,
                "bias": bias,
                "ones": ones,
            }
        )

    res = run_bass_kernel_spmd(
        nc, in_maps, core_ids=list(range(N_CORES)), trace=TRACE
    )
    LAST_EXEC_NS = res.exec_time_ns

    new_h = np.concatenate([res.results[k]["new_h"] for k in range(N_CORES)], axis=0)
    new_c = np.concatenate([res.results[k]["new_c"] for k in range(N_CORES)], axis=0)
    return new_h, new_c


# revision 15
# speedup vs baseline: 1.9671x; 1.7235x over previous
"""CoordinateLSTM cell on 8 Trainium2 NeuronCores (Bass/Tile, data-parallel).

Computes, for B=32768, I=H=128:
    total = concat([x, h], -1)                # [B, 256]
    s1 = sigmoid(total @ W1.T + b1)
    s2 = sigmoid(total @ W2.T + b2)
    fl = tanh   (total @ Wf.T + bf)
    s3 = sigmoid(total @ W3.T + b3)
    new_c = c * s1 + s2 * fl
    new_h = tanh(new_c) * s3

Sharding: batch dim split 8 ways (4096 rows/core); weights replicated.

Per-core kernel structure (per 1024-row chunk):
  - gpsimd (SWDGE) DMA loads x,h with an inline f32->bf16 cast (free cast)
  - HWDGE xbar DMA-transposes each 128-row subtile to feature-major layout
  - per 128-row subtile: 3 accumulating bf16 matmuls into one PSUM bank
    [128, 512]: xT.T@Wtx + hT.T@Wth + ones.T@bias (rank-1 bias add)
  - ScalarE sigmoid/tanh directly off PSUM (gates packed [s1|s2|s3|fl])
  - VectorE elementwise combine in f32, stores via HWDGE
"""

import sys

if "/opt/trn_rl_repo" not in sys.path:
    sys.path.insert(0, "/opt/trn_rl_repo")

import numpy as np
import ml_dtypes

BF16 = ml_dtypes.bfloat16
MM_DT = np.float16  # matmul operand dtype: fp16 = 10-bit mantissa, 1 cyc/row

B, I, H = 32768, 128, 128
N_CORES = 8
B_CORE = B // N_CORES  # 4096
SUB = 128              # rows per matmul tile (M)
G = 512                # stacked gate width: [s1 | s2 | s3 | fl]
SUBS_PER_GROUP = 4     # subtiles per PSUM group (4 banks)
GROUPS_PER_CHUNK = 2
CHUNK = SUB * SUBS_PER_GROUP * GROUPS_PER_CHUNK  # 1024 rows, load granularity

TRACE = False          # set by test.py to profile
LAST_EXEC_NS = None

_cache = {}


def _build(rows, reps=1, loop_n=1):
    """Build + compile the per-core Bass program for `rows` rows.

    reps > 1 unrolls the whole computation that many times; loop_n > 1 wraps
    it in a device-side For_i loop. Both are idempotent (same inputs/outputs)
    and exist so wall-clock differencing can recover the pure kernel
    execution time without NTFF profiling.
    """
    import concourse.bacc as bacc
    import concourse.bass as bass
    import concourse.tile as tile
    import concourse.mybir as mybir
    from contextlib import ExitStack, nullcontext

    dt = mybir.dt
    global MM_DT_BIR
    MM_DT_BIR = dt.float16 if MM_DT == np.float16 else dt.bfloat16
    AF = mybir.ActivationFunctionType
    assert rows % CHUNK == 0
    n_chunks = rows // CHUNK
    spc = CHUNK // SUB  # subtiles per chunk = 8

    nc = bacc.Bacc(
        "TRN2",
        target_bir_lowering=False,
        debug=False,
        enable_asserts=False,
        num_devices=N_CORES,
    )
    x_d = nc.dram_tensor("x", [rows, I], dt.float32, kind="ExternalInput")
    h_d = nc.dram_tensor("h", [rows, H], dt.float32, kind="ExternalInput")
    c_d = nc.dram_tensor("c", [rows, H], dt.float32, kind="ExternalInput")
    wtx_d = nc.dram_tensor("wtx", [I, G], MM_DT_BIR, kind="ExternalInput")
    wth_d = nc.dram_tensor("wth", [H, G], MM_DT_BIR, kind="ExternalInput")
    bias_d = nc.dram_tensor("bias", [1, G], MM_DT_BIR, kind="ExternalInput")
    ones_d = nc.dram_tensor("ones", [1, SUB], MM_DT_BIR, kind="ExternalInput")
    nh_d = nc.dram_tensor("new_h", [rows, H], dt.float32, kind="ExternalOutput")
    ncv_d = nc.dram_tensor("new_c", [rows, H], dt.float32, kind="ExternalOutput")

    # DRAM slab views: partition p holds `spc` CONSECUTIVE rows (contiguous
    # 4 KiB per partition -> 1 DMA descriptor per partition instead of 8).
    # Logical subtile r of a chunk is the strided row set {spc*p + r}; the
    # same mapping is applied to x, h, c and the outputs, so the matmul /
    # elementwise / store row-identity stays consistent.
    x_r = x_d[:].rearrange("(n p r) c -> n p r c", r=spc, p=SUB)
    h_r = h_d[:].rearrange("(n p r) c -> n p r c", r=spc, p=SUB)
    c_r = c_d[:].rearrange("(n p r) c -> n p r c", r=spc, p=SUB)
    nh_r = nh_d[:].rearrange("(n p r) c -> n p r c", r=spc, p=SUB)
    ncv_r = ncv_d[:].rearrange("(n p r) c -> n p r c", r=spc, p=SUB)

    with tile.TileContext(nc) as tc, ExitStack() as ctx:
        const = ctx.enter_context(tc.tile_pool(name="const", bufs=1))
        wtx_sb = const.tile([I, G], MM_DT_BIR)
        nc.sync.dma_start(wtx_sb[:], wtx_d[:])
        wth_sb = const.tile([H, G], MM_DT_BIR)
        nc.sync.dma_start(wth_sb[:], wth_d[:])
        bias_sb = const.tile([1, G], MM_DT_BIR)
        nc.sync.dma_start(bias_sb[:], bias_d[:])
        ones_sb = const.tile([1, SUB], MM_DT_BIR)
        nc.sync.dma_start(ones_sb[:], ones_d[:])

        xin = ctx.enter_context(tc.tile_pool(name="xin", bufs=2))
        xtp = ctx.enter_context(tc.tile_pool(name="xtp", bufs=2))
        cin = ctx.enter_context(tc.tile_pool(name="cin", bufs=2))
        psum = ctx.enter_context(
            tc.tile_pool(name="psum", bufs=2, space=bass.MemorySpace.PSUM)
        )
        sigp = ctx.enter_context(tc.tile_pool(name="sigp", bufs=3))
        post = ctx.enter_context(tc.tile_pool(name="post", bufs=3))

        loop_cm = tc.For_i(0, loop_n, 1) if loop_n > 1 else nullcontext()
        with loop_cm:
         for _rep in range(reps):
          for n in range(n_chunks):
            xbf = xin.tile([SUB, spc, I], MM_DT_BIR, tag="xbf")
            nc.gpsimd.dma_start(xbf[:], x_r[n])  # f32 -> bf16 cast in DMA
            hbf = xin.tile([SUB, spc, H], MM_DT_BIR, tag="hbf")
            nc.gpsimd.dma_start(hbf[:], h_r[n])
            c_sb = cin.tile([SUB, spc, H], dt.float32, tag="c")
            nc.sync.dma_start(c_sb[:], c_r[n])

            xT = xtp.tile([I, spc, SUB], MM_DT_BIR, tag="xT")
            hT = xtp.tile([H, spc, SUB], MM_DT_BIR, tag="hT")
            # One blocked xbar transpose per input: out[c, r, p] = in[p, r*128+c],
            # i.e. xT[:, r, :] is the transpose of x subtile r.
            nc.scalar.dma_start(xT[:], xbf[:], transpose=True)
            nc.scalar.dma_start(hT[:], hbf[:], transpose=True)

            ncw = post.tile([SUB, spc, H], dt.float32, tag="ncw")
            nhw = post.tile([SUB, spc, H], dt.float32, tag="nhw")
            for g in range(GROUPS_PER_CHUNK):
                ps = psum.tile([SUB, SUBS_PER_GROUP, G], dt.float32, tag="ps")
                for i in range(SUBS_PER_GROUP):
                    s = g * SUBS_PER_GROUP + i
                    nc.tensor.matmul(
                        ps[:, i, :], xT[:, s, :], wtx_sb[:], start=True, stop=False
                    )
                    nc.tensor.matmul(
                        ps[:, i, :], hT[:, s, :], wth_sb[:], start=False, stop=False
                    )
                    nc.tensor.matmul(
                        ps[:, i, :], ones_sb[:], bias_sb[:], start=False, stop=True
                    )

                sig = sigp.tile([SUB, SUBS_PER_GROUP, G], dt.float32, tag="sig")
                nc.scalar.activation(sig[:, :, 0:384], ps[:, :, 0:384], AF.Sigmoid)
                nc.scalar.activation(sig[:, :, 384:512], ps[:, :, 384:512], AF.Tanh)

                gsl = slice(g * SUBS_PER_GROUP, (g + 1) * SUBS_PER_GROUP)
                c_sl = c_sb[:, gsl, :]
                ncw_sl = ncw[:, gsl, :]
                nhw_sl = nhw[:, gsl, :]
                m1 = post.tile([SUB, SUBS_PER_GROUP, H], dt.float32, tag="m1")
                nc.vector.tensor_mul(m1[:], c_sl, sig[:, :, 0:128])
                m2 = post.tile([SUB, SUBS_PER_GROUP, H], dt.float32, tag="m2")
                nc.vector.tensor_mul(m2[:], sig[:, :, 128:256], sig[:, :, 384:512])
                nc.vector.tensor_add(ncw_sl, m1[:], m2[:])
                th = post.tile([SUB, SUBS_PER_GROUP, H], dt.float32, tag="th")
                nc.scalar.activation(th[:], ncw_sl, AF.Tanh)
                nc.vector.tensor_mul(nhw_sl, th[:], sig[:, :, 256:384])

            nc.sync.dma_start(ncv_r[n], ncw[:])
            nc.sync.dma_start(nh_r[n], nhw[:])

    nc.compile()
    return nc


def _get_program(rows):
    if rows not in _cache:
        _cache[rows] = _build(rows)
    return _cache[rows]


def _host_prep(W1, b1, W2, b2, Wf, bf, W3, b3):
    # Gate packing along the 512-wide output dim: [s1 | s2 | s3 | fl] so the
    # three sigmoid gates are contiguous for one ScalarE op.
    wtx = np.concatenate(
        [W1[:, :I].T, W2[:, :I].T, W3[:, :I].T, Wf[:, :I].T], axis=1
    ).astype(MM_DT)
    wth = np.concatenate(
        [W1[:, I:].T, W2[:, I:].T, W3[:, I:].T, Wf[:, I:].T], axis=1
    ).astype(MM_DT)
    bias = np.concatenate([b1, b2, b3, bf]).reshape(1, G).astype(MM_DT)
    ones = np.ones((1, SUB), MM_DT)
    return wtx, wth, bias, ones


def _make_runner(nc):
    """Cached jitted SPMD executor for `nc` (mirrors bass2jax.run_bass_via_pjrt
    but without output-buffer donation so device-resident inputs can be reused
    across timing calls)."""
    import jax
    import concourse.mybir as mybir
    from jax.experimental.shard_map import shard_map
    from jax.sharding import Mesh, PartitionSpec
    from concourse.bass2jax import (
        _bass_exec_p,
        install_neuronx_cc_hook,
        partition_id_tensor,
    )

    install_neuronx_cc_hook()
    assert nc.dbg_addr is None
    partition_name = nc.partition_id_tensor.name if nc.partition_id_tensor else None

    in_names, out_names, out_avals, zero_outs = [], [], [], []
    for alloc in nc.m.functions[0].allocations:
        if not isinstance(alloc, mybir.MemoryLocationSet):
            continue
        name = alloc.memorylocations[0].name
        if alloc.kind == "ExternalInput":
            if name != partition_name:
                in_names.append(name)
        elif alloc.kind == "ExternalOutput":
            out_names.append(name)
            shape = tuple(alloc.tensor_shape)
            dtype = mybir.dt.np(alloc.dtype)
            out_avals.append(jax.core.ShapedArray(shape, dtype))
            zero_outs.append(np.zeros(shape, dtype))
    n_params = len(in_names)
    all_names = in_names + out_names
    if partition_name is not None:
        all_names = all_names + [partition_name]

    def _body(*args):
        operands = list(args)
        if partition_name is not None:
            operands.append(partition_id_tensor())
        outs = _bass_exec_p.bind(
            *operands,
            out_avals=tuple(out_avals),
            in_names=tuple(all_names),
            out_names=tuple(out_names),
            lowering_input_output_aliases=(),
            sim_require_finite=True,
            sim_require_nnan=True,
            nc=nc,
        )
        return tuple(outs)

    devices = jax.devices()[:N_CORES]
    mesh = Mesh(np.asarray(devices), ("core",))
    n_all = n_params + len(out_names)
    sharded = jax.jit(
        shard_map(
            _body,
            mesh=mesh,
            in_specs=(PartitionSpec("core"),) * n_all,
            out_specs=(PartitionSpec("core"),) * len(out_names),
            check_rep=False,
        ),
        keep_unused=True,
    )
    return sharded, in_names, out_names, zero_outs


def _stage_inputs(in_maps, in_names, zero_outs):
    import jax

    concat_in = [
        np.concatenate([m[name] for m in in_maps], axis=0) for name in in_names
    ]
    concat_zeros = [
        np.zeros((N_CORES * z.shape[0], *z.shape[1:]), z.dtype) for z in zero_outs
    ]
    return [jax.device_put(a) for a in concat_in + concat_zeros]


def bench(
    x, h, c, W1, b1, W2, b2, Wf, bf, W3, b3, loop_lo=2048, loop_hi=6144, n_calls=4
):
    """Measure per-invocation HW time via wall-clock differencing between two
    device-side-looped builds (loop_lo vs loop_hi iterations), which cancels
    the per-call dispatch overhead. Returns (kernel_ns, tlo_list, thi_list)."""
    import time as _time

    import jax

    x = np.ascontiguousarray(x, np.float32)
    h = np.ascontiguousarray(h, np.float32)
    c = np.ascontiguousarray(c, np.float32)
    wtx, wth, bias, ones = _host_prep(W1, b1, W2, b2, Wf, bf, W3, b3)
    rows = x.shape[0] // N_CORES
    in_maps = []
    for k in range(N_CORES):
        sl = slice(k * rows, (k + 1) * rows)
        in_maps.append(
            dict(x=x[sl], h=h[sl], c=c[sl], wtx=wtx, wth=wth, bias=bias, ones=ones)
        )

    results = {}
    for loop_n in (loop_lo, loop_hi):
        nc = _build(rows, loop_n=loop_n)
        sharded, in_names, out_names, zero_outs = _make_runner(nc)
        dev_args = _stage_inputs(in_maps, in_names, zero_outs)
        outs = sharded(*dev_args)  # warmup/compile
        jax.block_until_ready(outs)
        times = []
        for _ in range(n_calls):
            t0 = _time.perf_counter()
            outs = sharded(*dev_args)
            jax.block_until_ready(outs)
            times.append((_time.perf_counter() - t0) * 1e9)
        results[loop_n] = times
    tlo = min(results[loop_lo])
    thi = min(results[loop_hi])
    kernel_ns = (thi - tlo) / (loop_hi - loop_lo)
    return kernel_ns, results[loop_lo], results[loop_hi]


def kernel(x, h, c, W1, b1, W2, b2, Wf, bf, W3, b3):
    from concourse.bass_utils import run_bass_kernel_spmd

    global LAST_EXEC_NS
    x = np.ascontiguousarray(x, np.float32)
    h = np.ascontiguousarray(h, np.float32)
    c = np.ascontiguousarray(c, np.float32)
    wtx, wth, bias, ones = _host_prep(W1, b1, W2, b2, Wf, bf, W3, b3)

    rows = x.shape[0] // N_CORES
    nc = _get_program(rows)

    in_maps = []
    for k in range(N_CORES):
        sl = slice(k * rows, (k + 1) * rows)
        in_maps.append(
            {
                "x": x[sl],
                "h": h[sl],
                "c": c[sl],
                "wtx": wtx,
                "wth": wth,
                "bias": bias,
                "ones": ones,
            }
        )

    res = run_bass_kernel_spmd(
        nc, in_maps, core_ids=list(range(N_CORES)), trace=TRACE
    )
    LAST_EXEC_NS = res.exec_time_ns

    new_h = np.concatenate([res.results[k]["new_h"] for k in range(N_CORES)], axis=0)
    new_c = np.concatenate([res.results[k]["new_c"] for k in range(N_CORES)], axis=0)
    return new_h, new_c


# revision 27
# speedup vs baseline: 2.3737x; 1.2067x over previous
"""CoordinateLSTM cell on 8 Trainium2 NeuronCores (Bass/Tile, data-parallel).

Computes, for B=32768, I=H=128:
    total = concat([x, h], -1)                # [B, 256]
    s1 = sigmoid(total @ W1.T + b1)
    s2 = sigmoid(total @ W2.T + b2)
    fl = tanh   (total @ Wf.T + bf)
    s3 = sigmoid(total @ W3.T + b3)
    new_c = c * s1 + s2 * fl
    new_h = tanh(new_c) * s3

Sharding: batch dim split 8 ways (4096 rows/core); weights replicated.

Per-core kernel structure (per 1024-row chunk):
  - gpsimd (SWDGE) DMA loads x,h with an inline f32->bf16 cast (free cast)
  - HWDGE xbar DMA-transposes each 128-row subtile to feature-major layout
  - per 128-row subtile: 3 accumulating bf16 matmuls into one PSUM bank
    [128, 512]: xT.T@Wtx + hT.T@Wth + ones.T@bias (rank-1 bias add)
  - ScalarE sigmoid/tanh directly off PSUM (gates packed [s1|s2|s3|fl])
  - VectorE elementwise combine in f32, stores via HWDGE
"""

import sys

if "/opt/trn_rl_repo" not in sys.path:
    sys.path.insert(0, "/opt/trn_rl_repo")

import numpy as np
import ml_dtypes

BF16 = ml_dtypes.bfloat16
MM_DT = np.float16  # matmul operand dtype: fp16 = 10-bit mantissa, 1 cyc/row

B, I, H = 32768, 128, 128
N_CORES = 8
B_CORE = B // N_CORES  # 4096
SUB = 128              # rows per matmul tile (M)
G = 512                # stacked gate width: [s1 | s2 | s3 | fl]
SUBS_PER_GROUP = 4     # subtiles per PSUM group (4 banks)
GROUPS_PER_CHUNK = 2
CHUNK = SUB * SUBS_PER_GROUP * GROUPS_PER_CHUNK  # 1024 rows, load granularity

TRACE = False          # set by test.py to profile
LAST_EXEC_NS = None
BIAS_MM = True         # dev knob: emit the rank-1 bias matmul (timing A/B)
WARMUP = 8             # dev knob: number of PE warmup matmuls

_cache = {}


def _build(rows, reps=1, loop_n=1):
    """Build + compile the per-core Bass program for `rows` rows.

    reps > 1 unrolls the whole computation that many times; loop_n > 1 wraps
    it in a device-side For_i loop. Both are idempotent (same inputs/outputs)
    and exist so wall-clock differencing can recover the pure kernel
    execution time without NTFF profiling.
    """
    import concourse.bacc as bacc
    import concourse.bass as bass
    import concourse.tile as tile
    import concourse.mybir as mybir
    from contextlib import ExitStack, nullcontext

    dt = mybir.dt
    global MM_DT_BIR
    MM_DT_BIR = dt.float16 if MM_DT == np.float16 else dt.bfloat16
    AF = mybir.ActivationFunctionType
    assert rows % CHUNK == 0
    n_chunks = rows // CHUNK
    spc = CHUNK // SUB  # subtiles per chunk

    nc = bacc.Bacc(
        "TRN2",
        target_bir_lowering=False,
        debug=False,
        enable_asserts=False,
        num_devices=N_CORES,
    )
    x_d = nc.dram_tensor("x", [rows, I], dt.float32, kind="ExternalInput")
    h_d = nc.dram_tensor("h", [rows, H], dt.float32, kind="ExternalInput")
    c_d = nc.dram_tensor("c", [rows, H], dt.float32, kind="ExternalInput")
    wtx_d = nc.dram_tensor("wtx", [I, G], MM_DT_BIR, kind="ExternalInput")
    wth_d = nc.dram_tensor("wth", [H, G], MM_DT_BIR, kind="ExternalInput")
    bias_d = nc.dram_tensor("bias", [1, G], MM_DT_BIR, kind="ExternalInput")
    ones_d = nc.dram_tensor("ones", [1, SUB], MM_DT_BIR, kind="ExternalInput")
    nh_d = nc.dram_tensor("new_h", [rows, H], dt.float32, kind="ExternalOutput")
    ncv_d = nc.dram_tensor("new_c", [rows, H], dt.float32, kind="ExternalOutput")

    # DRAM slab views: partition p holds `spc` CONSECUTIVE rows (contiguous
    # 4 KiB per partition -> 1 DMA descriptor per partition instead of 8).
    # Logical subtile r of a chunk is the strided row set {spc*p + r}; the
    # same mapping is applied to x, h, c and the outputs, so the matmul /
    # elementwise / store row-identity stays consistent.
    x_r = x_d[:].rearrange("(n p r) c -> n p r c", r=spc, p=SUB)
    h_r = h_d[:].rearrange("(n p r) c -> n p r c", r=spc, p=SUB)
    c_r = c_d[:].rearrange("(n p r) c -> n p r c", r=spc, p=SUB)
    nh_r = nh_d[:].rearrange("(n p r) c -> n p r c", r=spc, p=SUB)
    ncv_r = ncv_d[:].rearrange("(n p r) c -> n p r c", r=spc, p=SUB)

    with tile.TileContext(nc) as tc, ExitStack() as ctx:
        const = ctx.enter_context(tc.tile_pool(name="const", bufs=1))
        wtx_sb = const.tile([I, G], MM_DT_BIR)
        nc.sync.dma_start(wtx_sb[:], wtx_d[:])
        wth_sb = const.tile([H, G], MM_DT_BIR)
        nc.sync.dma_start(wth_sb[:], wth_d[:])
        bias_sb = const.tile([1, G], MM_DT_BIR)
        nc.sync.dma_start(bias_sb[:], bias_d[:])
        ones_sb = const.tile([1, SUB], MM_DT_BIR)
        nc.sync.dma_start(ones_sb[:], ones_d[:])

        xin = ctx.enter_context(tc.tile_pool(name="xin", bufs=2))
        xtp = ctx.enter_context(tc.tile_pool(name="xtp", bufs=2))
        cin = ctx.enter_context(tc.tile_pool(name="cin", bufs=2))
        psum = ctx.enter_context(
            tc.tile_pool(name="psum", bufs=2, space=bass.MemorySpace.PSUM)
        )
        sigp = ctx.enter_context(tc.tile_pool(name="sigp", bufs=3))
        post = ctx.enter_context(tc.tile_pool(name="post", bufs=3))

        # Zero tile for PE warmup matmuls (contents irrelevant).
        wu = const.tile([SUB, G], MM_DT_BIR)
        nc.gpsimd.memset(wu[:], 0.0)

        # Dummy activation at t=0: walrus inserts the sigmoid/tanh ACT table
        # load right before the first Activation on the ScalarE stream, so
        # this hoists the ~2.6 us table load into the DMA fill phase instead
        # of the first real sigmoid's critical path.
        actwarm = const.tile([1, 1], dt.float32)
        nc.scalar.activation(actwarm[:], wu[0:1, 0:1], AF.Sigmoid)

        loop_cm = tc.For_i(0, loop_n, 1) if loop_n > 1 else nullcontext()
        with loop_cm:
         for _rep in range(reps):
          # PE warmup: input-independent matmuls run at t=0, overlapping the
          # DMA fill, so the HAM clock-gate reaches 2.4 GHz before the real
          # matmuls start (~3.4 us of sustained PE activity required).
          if WARMUP:
            ps_w = psum.tile([SUB, SUBS_PER_GROUP, G], dt.float32, tag="ps")
            for _w in range(WARMUP):
                nc.tensor.matmul(
                    ps_w[:, 0, :], wu[:, 0:SUB], wu[:], start=True, stop=True
                )
          for n in range(n_chunks):
            xbf = xin.tile([SUB, spc, I], MM_DT_BIR, tag="xbf")
            nc.gpsimd.dma_start(xbf[:], x_r[n])  # f32 -> bf16 cast in DMA
            hbf = xin.tile([SUB, spc, H], MM_DT_BIR, tag="hbf")
            nc.gpsimd.dma_start(hbf[:], h_r[n])
            c_sb = cin.tile([SUB, spc, H], dt.float32, tag="c")
            nc.sync.dma_start(c_sb[:], c_r[n])

            xT = xtp.tile([I, spc, SUB], MM_DT_BIR, tag="xT")
            hT = xtp.tile([H, spc, SUB], MM_DT_BIR, tag="hT")
            # One blocked xbar transpose per input: out[c, r, p] = in[p, r*128+c],
            # i.e. xT[:, r, :] is the transpose of x subtile r.
            nc.scalar.dma_start(xT[:], xbf[:], transpose=True)
            nc.scalar.dma_start(hT[:], hbf[:], transpose=True)

            ncw = post.tile([SUB, spc, H], dt.float32, tag="ncw")
            nhw = post.tile([SUB, spc, H], dt.float32, tag="nhw")
            for g in range(GROUPS_PER_CHUNK):
                ps = psum.tile([SUB, SUBS_PER_GROUP, G], dt.float32, tag="ps")
                # All x-parts first: they only need xT, so the PE's in-order
                # stream isn't blocked on the hT transpose during the fill.
                for i in range(SUBS_PER_GROUP):
                    s = g * SUBS_PER_GROUP + i
                    nc.tensor.matmul(
                        ps[:, i, :], xT[:, s, :], wtx_sb[:], start=True, stop=False
                    )
                for i in range(SUBS_PER_GROUP):
                    s = g * SUBS_PER_GROUP + i
                    nc.tensor.matmul(
                        ps[:, i, :], hT[:, s, :], wth_sb[:], start=False,
                        stop=not BIAS_MM,
                    )
                if BIAS_MM:
                    for i in range(SUBS_PER_GROUP):
                        nc.tensor.matmul(
                            ps[:, i, :], ones_sb[:], bias_sb[:], start=False, stop=True
                        )

                sig = sigp.tile([SUB, SUBS_PER_GROUP, G], dt.float32, tag="sig")
                nc.scalar.activation(sig[:, :, 0:384], ps[:, :, 0:384], AF.Sigmoid)
                nc.scalar.activation(sig[:, :, 384:512], ps[:, :, 384:512], AF.Tanh)

                gsl = slice(g * SUBS_PER_GROUP, (g + 1) * SUBS_PER_GROUP)
                c_sl = c_sb[:, gsl, :]
                ncw_sl = ncw[:, gsl, :]
                nhw_sl = nhw[:, gsl, :]
                m1 = post.tile([SUB, SUBS_PER_GROUP, H], dt.float32, tag="m1")
                nc.vector.tensor_mul(m1[:], c_sl, sig[:, :, 0:128])
                m2 = post.tile([SUB, SUBS_PER_GROUP, H], dt.float32, tag="m2")
                nc.vector.tensor_mul(m2[:], sig[:, :, 128:256], sig[:, :, 384:512])
                nc.vector.tensor_add(ncw_sl, m1[:], m2[:])
                th = post.tile([SUB, SUBS_PER_GROUP, H], dt.float32, tag="th")
                nc.scalar.activation(th[:], ncw_sl, AF.Tanh)
                nc.vector.tensor_mul(nhw_sl, th[:], sig[:, :, 256:384])

            nc.sync.dma_start(ncv_r[n], ncw[:])
            nc.sync.dma_start(nh_r[n], nhw[:])

    nc.compile()
    return nc


def _get_program(rows):
    if rows not in _cache:
        _cache[rows] = _build(rows)
    return _cache[rows]


def _host_prep(W1, b1, W2, b2, Wf, bf, W3, b3):
    # Gate packing along the 512-wide output dim: [s1 | s2 | s3 | fl] so the
    # three sigmoid gates are contiguous for one ScalarE op.
    wtx = np.concatenate(
        [W1[:, :I].T, W2[:, :I].T, W3[:, :I].T, Wf[:, :I].T], axis=1
    ).astype(MM_DT)
    wth = np.concatenate(
        [W1[:, I:].T, W2[:, I:].T, W3[:, I:].T, Wf[:, I:].T], axis=1
    ).astype(MM_DT)
    bias = np.concatenate([b1, b2, b3, bf]).reshape(1, G).astype(MM_DT)
    ones = np.ones((1, SUB), MM_DT)
    return wtx, wth, bias, ones


def _make_runner(nc):
    """Cached jitted SPMD executor for `nc` (mirrors bass2jax.run_bass_via_pjrt
    but without output-buffer donation so device-resident inputs can be reused
    across timing calls)."""
    import jax
    import concourse.mybir as mybir
    from jax.experimental.shard_map import shard_map
    from jax.sharding import Mesh, PartitionSpec
    from concourse.bass2jax import (
        _bass_exec_p,
        install_neuronx_cc_hook,
        partition_id_tensor,
    )

    install_neuronx_cc_hook()
    assert nc.dbg_addr is None
    partition_name = nc.partition_id_tensor.name if nc.partition_id_tensor else None

    in_names, out_names, out_avals, zero_outs = [], [], [], []
    for alloc in nc.m.functions[0].allocations:
        if not isinstance(alloc, mybir.MemoryLocationSet):
            continue
        name = alloc.memorylocations[0].name
        if alloc.kind == "ExternalInput":
            if name != partition_name:
                in_names.append(name)
        elif alloc.kind == "ExternalOutput":
            out_names.append(name)
            shape = tuple(alloc.tensor_shape)
            dtype = mybir.dt.np(alloc.dtype)
            out_avals.append(jax.core.ShapedArray(shape, dtype))
            zero_outs.append(np.zeros(shape, dtype))
    n_params = len(in_names)
    all_names = in_names + out_names
    if partition_name is not None:
        all_names = all_names + [partition_name]

    def _body(*args):
        operands = list(args)
        if partition_name is not None:
            operands.append(partition_id_tensor())
        outs = _bass_exec_p.bind(
            *operands,
            out_avals=tuple(out_avals),
            in_names=tuple(all_names),
            out_names=tuple(out_names),
            lowering_input_output_aliases=(),
            sim_require_finite=True,
            sim_require_nnan=True,
            nc=nc,
        )
        return tuple(outs)

    devices = jax.devices()[:N_CORES]
    mesh = Mesh(np.asarray(devices), ("core",))
    n_all = n_params + len(out_names)
    sharded = jax.jit(
        shard_map(
            _body,
            mesh=mesh,
            in_specs=(PartitionSpec("core"),) * n_all,
            out_specs=(PartitionSpec("core"),) * len(out_names),
            check_rep=False,
        ),
        keep_unused=True,
    )
    return sharded, in_names, out_names, zero_outs


def _stage_inputs(in_maps, in_names, zero_outs):
    import jax

    concat_in = [
        np.concatenate([m[name] for m in in_maps], axis=0) for name in in_names
    ]
    concat_zeros = [
        np.zeros((N_CORES * z.shape[0], *z.shape[1:]), z.dtype) for z in zero_outs
    ]
    return [jax.device_put(a) for a in concat_in + concat_zeros]


def bench(
    x, h, c, W1, b1, W2, b2, Wf, bf, W3, b3, loop_lo=2048, loop_hi=6144, n_calls=4
):
    """Measure per-invocation HW time via wall-clock differencing between two
    device-side-looped builds (loop_lo vs loop_hi iterations), which cancels
    the per-call dispatch overhead. Returns (kernel_ns, tlo_list, thi_list)."""
    import time as _time

    import jax

    x = np.ascontiguousarray(x, np.float32)
    h = np.ascontiguousarray(h, np.float32)
    c = np.ascontiguousarray(c, np.float32)
    wtx, wth, bias, ones = _host_prep(W1, b1, W2, b2, Wf, bf, W3, b3)
    rows = x.shape[0] // N_CORES
    in_maps = []
    for k in range(N_CORES):
        sl = slice(k * rows, (k + 1) * rows)
        in_maps.append(
            dict(x=x[sl], h=h[sl], c=c[sl], wtx=wtx, wth=wth, bias=bias, ones=ones)
        )

    results = {}
    for loop_n in (loop_lo, loop_hi):
        nc = _build(rows, loop_n=loop_n)
        sharded, in_names, out_names, zero_outs = _make_runner(nc)
        dev_args = _stage_inputs(in_maps, in_names, zero_outs)
        outs = sharded(*dev_args)  # warmup/compile
        jax.block_until_ready(outs)
        times = []
        for _ in range(n_calls):
            t0 = _time.perf_counter()
            outs = sharded(*dev_args)
            jax.block_until_ready(outs)
            times.append((_time.perf_counter() - t0) * 1e9)
        results[loop_n] = times
    tlo = min(results[loop_lo])
    thi = min(results[loop_hi])
    kernel_ns = (thi - tlo) / (loop_hi - loop_lo)
    return kernel_ns, results[loop_lo], results[loop_hi]


def kernel(x, h, c, W1, b1, W2, b2, Wf, bf, W3, b3):
    from concourse.bass_utils import run_bass_kernel_spmd

    global LAST_EXEC_NS
    x = np.ascontiguousarray(x, np.float32)
    h = np.ascontiguousarray(h, np.float32)
    c = np.ascontiguousarray(c, np.float32)
    wtx, wth, bias, ones = _host_prep(W1, b1, W2, b2, Wf, bf, W3, b3)

    rows = x.shape[0] // N_CORES
    nc = _get_program(rows)

    in_maps = []
    for k in range(N_CORES):
        sl = slice(k * rows, (k + 1) * rows)
        in_maps.append(
            {
                "x": x[sl],
                "h": h[sl],
                "c": c[sl],
                "wtx": wtx,
                "wth": wth,
                "bias": bias,
                "ones": ones,
            }
        )

    res = run_bass_kernel_spmd(
        nc, in_maps, core_ids=list(range(N_CORES)), trace=TRACE
    )
    LAST_EXEC_NS = res.exec_time_ns

    new_h = np.concatenate([res.results[k]["new_h"] for k in range(N_CORES)], axis=0)
    new_c = np.concatenate([res.results[k]["new_c"] for k in range(N_CORES)], axis=0)
    return new_h, new_c
